# revision 6
# baseline (speedup 1.0000x reference)
"""Trainium2 Bass kernel for segment-reduce attention module.

reference:
    proj = embedding @ W                                   [T, D]
    seg_sum = segment_sum(proj, obj)                       [N, D]
    counts = segment_sum(ones, obj)                        [N]
    tg = tanh(seg_sum / max(counts, 1))                    [N, D]
    scores = sigmoid(sum(embedding * tg[obj], -1))         [T]
    rep = segment_sum(embedding * scores[:, None], obj)    [N, D]
    return rep[obj]                                        [T, D]

Key identities exploited:
  - segment_sum(emb @ W) == segment_sum(emb) @ W: the [T,D]@[D,D] matmul
    collapses to [N,D]@[D,D].
  - segment_sum(emb * s) == (A * s)^T @ emb: scale the one-hot matrix
    (128-wide stream) instead of the embeddings (256-wide stream).

Sharding: tokens are sorted by segment. Segments are partitioned into 64
blocks of 128 segments; each of the 8 cores owns 8 consecutive blocks.
No cross-core communication.

Per block the tokens are padded to L = NT*128 and token (p, k) of the
on-device layout is original token p*NT + k, making every DMA a plain
contiguous 2D slice. One-hot matrices A [tok, seg] / AT [seg, tok] are
host-built in bf16 (exact) and drive all segment reductions and
broadcasts as TensorEngine matmuls with fp32 PSUM accumulation.
"""

import sys

if "/opt/trn_rl_repo" not in sys.path:
    sys.path.insert(0, "/opt/trn_rl_repo")

import numpy as np
import ml_dtypes

from concourse import bacc, mybir
import concourse.bass as bass
import concourse.tile as tile
from concourse.masks import make_identity

BF16 = ml_dtypes.bfloat16

T = 524288
D = 256
N_SEG = 8192
N_CORES = 8
SEGB = 128                      # segments per block
N_BLOCKS = N_SEG // SEGB        # 64 total
BLOCKS_PER_CORE = N_BLOCKS // N_CORES  # 8
DA = D + 2                      # emb + ones column + pad (even stride)
OUT_CHUNK = 32                  # tiles per output staging DMA


def build_nc(NT: int, repeat: int = 1):
    """Build the per-core Bass graph. NT = 128-token tiles per block.
    NT must be even (pairs of tiles share one PSUM bank)."""
    assert NT % 2 == 0
    nc = bacc.Bacc()
    fp32 = mybir.dt.float32
    bf16 = mybir.dt.bfloat16
    B = BLOCKS_PER_CORE
    ACT = mybir.ActivationFunctionType

    emb_ext = nc.declare_dram_parameter("emb", [B * 128, NT * DA], bf16, isOutput=False)
    a_ext = nc.declare_dram_parameter("amat", [B * 128, NT * 128], bf16, isOutput=False)
    at_ext = nc.declare_dram_parameter("atmat", [B * 128, NT * 128], bf16, isOutput=False)
    w_ext = nc.declare_dram_parameter("w", [128, 2 * D], fp32, isOutput=False)
    out_ext = nc.declare_dram_parameter("out", [B * 128, NT * D], bf16, isOutput=True)

    n_chunks = (NT + OUT_CHUNK - 1) // OUT_CHUNK

    with tile.TileContext(nc) as tc:
        with (
            tc.tile_pool(name="const", bufs=1) as const_pool,
            tc.tile_pool(name="emb", bufs=2) as emb_pool,
            tc.tile_pool(name="amat", bufs=2) as a_pool,
            tc.tile_pool(name="atmat", bufs=2) as at_pool,
            tc.tile_pool(name="small", bufs=2) as small_pool,
            tc.tile_pool(name="prodscr", bufs=3) as prod_pool,
            tc.tile_pool(name="redscr", bufs=3) as red_pool,
            tc.tile_pool(name="ascaled", bufs=3) as as_pool,
            tc.tile_pool(name="outstage", bufs=2) as out_pool,
            tc.tile_pool(name="ps_segE", bufs=1, space="PSUM") as ps_segE,
            tc.tile_pool(name="ps_tr", bufs=1, space="PSUM") as ps_tr,
            tc.tile_pool(name="ps_tg", bufs=1, space="PSUM") as ps_tg,
            tc.tile_pool(name="ps_tgtok", bufs=2, space="PSUM") as ps_tgtok,
            tc.tile_pool(name="ps_rep", bufs=1, space="PSUM") as ps_rep,
            tc.tile_pool(name="ps_out", bufs=2, space="PSUM") as ps_out,
        ):
            w_sb = const_pool.tile([128, 2 * D], fp32)
            nc.sync.dma_start(out=w_sb[:], in_=w_ext[:, :])
            ident = const_pool.tile([128, 128], fp32)
            make_identity(nc, ident[:])

            for rep_i in range(repeat):
                for b in range(B):
                    # ---- load block ----
                    emb_sb = emb_pool.tile([128, NT * DA], bf16)
                    nc.sync.dma_start(out=emb_sb[:], in_=emb_ext[b * 128:(b + 1) * 128, :])
                    a_sb = a_pool.tile([128, NT * 128], bf16)
                    nc.sync.dma_start(out=a_sb[:], in_=a_ext[b * 128:(b + 1) * 128, :])
                    at_sb = at_pool.tile([128, NT * 128], bf16)
                    nc.sync.dma_start(out=at_sb[:], in_=at_ext[b * 128:(b + 1) * 128, :])

                    # ---- P1: segE[s, 0:256] = sum emb, segE[s, 256] = count ----
                    segE = ps_segE.tile([128, DA], fp32)
                    for k in range(NT):
                        nc.tensor.matmul(
                            segE[:],
                            lhsT=a_sb[:, k * 128:(k + 1) * 128],
                            rhs=emb_sb[:, k * DA:(k + 1) * DA],
                            start=(k == 0),
                            stop=(k == NT - 1),
                        )

                    # ---- epilogue: tg = tanh((segE/max(cnt,1)) @ W) ----
                    cnt = small_pool.tile([128, 1], fp32, tag="cnt")
                    nc.vector.tensor_scalar_max(cnt[:], segE[:, D:D + 1], 1.0)
                    inv = small_pool.tile([128, 1], fp32, tag="inv")
                    nc.vector.reciprocal(inv[:], cnt[:])
                    segmean = small_pool.tile([128, D], fp32, tag="segmean")
                    nc.vector.tensor_scalar_mul(segmean[:], segE[:, 0:D], inv[:, :])

                    trp = ps_tr.tile([128, D], fp32)
                    nc.tensor.transpose(trp[:, 0:128], segmean[:, 0:128], ident[:])
                    nc.tensor.transpose(trp[:, 128:256], segmean[:, 128:256], ident[:])
                    segmean_t = small_pool.tile([128, D], fp32, tag="segmeant")
                    nc.vector.tensor_copy(segmean_t[:], trp[:])

                    tgp = ps_tg.tile([128, D], fp32)
                    for h in range(2):
                        nc.tensor.matmul(
                            tgp[:],
                            lhsT=segmean_t[:, h * 128:(h + 1) * 128],
                            rhs=w_sb[:, h * D:(h + 1) * D],
                            start=(h == 0),
                            stop=(h == 1),
                        )
                    tg_sb = small_pool.tile([128, D], bf16, tag="tg")
                    nc.scalar.activation(tg_sb[:], tgp[:], ACT.Tanh)

                    # ---- P2a: dots[t] = sum_d emb[t,d] * tg[seg(t),d] ----
                    dots = small_pool.tile([128, NT], fp32, tag="dots")
                    for k in range(0, NT, 2):
                        ttp = ps_tgtok.tile([128, 2 * D], fp32)
                        for t in range(2):
                            nc.tensor.matmul(
                                ttp[:, t * D:(t + 1) * D],
                                lhsT=at_sb[:, (k + t) * 128:(k + t + 1) * 128],
                                rhs=tg_sb[:],
                                start=True, stop=True,
                            )
                        pscr = prod_pool.tile([128, 2 * D], bf16)
                        emb2 = emb_sb[:, k * DA:(k + 2) * DA].rearrange(
                            "p (t c) -> p t c", t=2)[:, :, 0:D]
                        nc.vector.tensor_tensor(
                            out=pscr[:].rearrange("p (t c) -> p t c", t=2),
                            in0=emb2,
                            in1=ttp[:].rearrange("p (t c) -> p t c", t=2),
                            op=mybir.AluOpType.mult,
                        )
                        for t in range(2):
                            rscr = red_pool.tile([128, D], bf16)
                            nc.scalar.activation(
                                rscr[:], pscr[:, t * D:(t + 1) * D], ACT.Copy,
                                accum_out=dots[:, k + t:k + t + 1])
                    sig = small_pool.tile([128, NT], fp32, tag="sig")
                    nc.scalar.activation(sig[:], dots[:], ACT.Sigmoid)

                    # ---- P2b: rep[s] = (A*sig)^T @ emb ----
                    repp = ps_rep.tile([128, D], fp32)
                    for k in range(NT):
                        a_scaled = as_pool.tile([128, 128], bf16)
                        nc.scalar.activation(
                            a_scaled[:], a_sb[:, k * 128:(k + 1) * 128],
                            ACT.Copy, scale=sig[:, k:k + 1])
                        nc.tensor.matmul(
                            repp[:],
                            lhsT=a_scaled[:],
                            rhs=emb_sb[:, k * DA:k * DA + D],
                            start=(k == 0),
                            stop=(k == NT - 1),
                        )
                    rep_sb = small_pool.tile([128, D], bf16, tag="rep")
                    nc.vector.tensor_copy(rep_sb[:], repp[:])

                    # ---- P3: out[t] = rep[seg(t)] ----
                    for c in range(n_chunks):
                        k0 = c * OUT_CHUNK
                        k1 = min(k0 + OUT_CHUNK, NT)
                        ostage = out_pool.tile([128, OUT_CHUNK * D], bf16)
                        for k in range(k0, k1, 2):
                            outp = ps_out.tile([128, 2 * D], fp32)
                            for t in range(2):
                                nc.tensor.matmul(
                                    outp[:, t * D:(t + 1) * D],
                                    lhsT=at_sb[:, (k + t) * 128:(k + t + 1) * 128],
                                    rhs=rep_sb[:],
                                    start=True, stop=True,
                                )
                            nc.vector.tensor_copy(
                                ostage[:, (k - k0) * D:(k - k0 + 2) * D], outp[:])
                        nc.scalar.dma_start(
                            out=out_ext[b * 128:(b + 1) * 128, k0 * D:k1 * D],
                            in_=ostage[:, 0:(k1 - k0) * D],
                        )
    nc.finalize()
    return nc


def prep_inputs(embedding, W, obj_to_img):
    """Host-side shard + layout. Returns (in_maps, meta)."""
    emb = np.asarray(embedding, dtype=np.float32)
    W = np.asarray(W, dtype=np.float32)
    obj = np.asarray(obj_to_img).astype(np.int64)

    bounds = np.searchsorted(obj, np.arange(0, N_SEG + 1, SEGB))
    cnts = np.diff(bounds)                     # tokens per block [64]
    NT = int(np.ceil(max(int(cnts.max()), 1) / 128.0))
    if NT % 2:
        NT += 1
    L = NT * 128

    emb_bf = emb.astype(BF16)
    w_in = np.ascontiguousarray(W.reshape(2, 128, D).transpose(1, 0, 2).reshape(128, 2 * D))

    idx = np.arange(L).reshape(128, NT)        # p, k -> p*NT + k
    in_maps = []
    meta = {"L": L, "NT": NT, "bounds": bounds, "cnts": cnts}
    for core in range(N_CORES):
        emb_c = np.zeros((BLOCKS_PER_CORE, 128, NT * DA), dtype=BF16)
        a_c = np.zeros((BLOCKS_PER_CORE, 128, NT * 128), dtype=BF16)
        at_c = np.zeros((BLOCKS_PER_CORE, 128, NT * 128), dtype=BF16)
        for bi in range(BLOCKS_PER_CORE):
            blk = core * BLOCKS_PER_CORE + bi
            start, cnt = int(bounds[blk]), int(cnts[blk])
            valid = idx < cnt                   # [128, NT]
            src = start + np.minimum(idx, max(cnt - 1, 0))
            eb = np.zeros((128, NT, DA), dtype=BF16)
            eb[:, :, :D] = np.where(valid[:, :, None], emb_bf[src], BF16(0))
            eb[:, :, D] = valid.astype(BF16)
            emb_c[bi] = eb.reshape(128, NT * DA)
            segloc = np.where(valid, obj[src] - blk * SEGB, 999)  # [128, NT]
            a_blk = (segloc[:, :, None] == np.arange(SEGB)[None, None, :])  # [p,k,s]
            a_c[bi] = a_blk.astype(BF16).reshape(128, NT * 128)
            at_c[bi] = np.ascontiguousarray(
                a_blk.transpose(2, 1, 0)).astype(BF16).reshape(128, NT * 128)
        in_maps.append({
            "emb": emb_c.reshape(BLOCKS_PER_CORE * 128, NT * DA),
            "amat": a_c.reshape(BLOCKS_PER_CORE * 128, NT * 128),
            "atmat": at_c.reshape(BLOCKS_PER_CORE * 128, NT * 128),
            "w": w_in,
        })
    return in_maps, meta


def unshard_output(core_outs, meta):
    """core_outs: list over cores of [B*128, NT*D] (bf16). -> [T, D] f32."""
    L, NT = meta["L"], meta["NT"]
    bounds, cnts = meta["bounds"], meta["cnts"]
    out = np.empty((T, D), dtype=np.float32)
    idx = np.arange(L).reshape(128, NT)
    for core in range(N_CORES):
        o = np.asarray(core_outs[core]).astype(np.float32)
        o = o.reshape(BLOCKS_PER_CORE, 128, NT, D)
        for bi in range(BLOCKS_PER_CORE):
            blk = core * BLOCKS_PER_CORE + bi
            start, cnt = int(bounds[blk]), int(cnts[blk])
            valid = idx < cnt
            p_i, k_i = np.nonzero(valid)
            out[start + idx[valid]] = o[bi, p_i, k_i]
    return out


def kernel(embedding, W, obj_to_img, num_segments):
    assert int(num_segments) == N_SEG
    in_maps, meta = prep_inputs(embedding, W, obj_to_img)
    nc = build_nc(meta["NT"])

    from concourse.bass_utils import run_bass_kernel_spmd
    res = run_bass_kernel_spmd(nc, in_maps, list(range(N_CORES)))
    core_outs = [res.results[i]["out"] for i in range(N_CORES)]
    return unshard_output(core_outs, meta)


# revision 7
# speedup vs baseline: 4.7797x; 4.7797x over previous
"""Trainium2 Bass kernel for segment-reduce attention module.

reference:
    proj = embedding @ W                                   [T, D]
    seg_sum = segment_sum(proj, obj)                       [N, D]
    counts = segment_sum(ones, obj)                        [N]
    tg = tanh(seg_sum / max(counts, 1))                    [N, D]
    scores = sigmoid(sum(embedding * tg[obj], -1))         [T]
    rep = segment_sum(embedding * scores[:, None], obj)    [N, D]
    return rep[obj]                                        [T, D]

Key identities exploited:
  - segment_sum(emb @ W) == segment_sum(emb) @ W: the [T,D]@[D,D] matmul
    collapses to [N,D]@[D,D].
  - segment_sum(emb * s) == (A * s)^T @ emb: scale the one-hot matrix
    (128-wide stream) instead of the embeddings (256-wide stream).

Sharding: tokens are sorted by segment. Segments are partitioned into 64
blocks of 128 segments; each of the 8 cores owns 8 consecutive blocks.
No cross-core communication.

Per block the tokens are padded to L = NT*128 and token (p, k) of the
on-device layout is original token p*NT + k, making every DMA a plain
contiguous 2D slice. One-hot matrices A [tok, seg] / AT [seg, tok] are
host-built in bf16 (exact) and drive all segment reductions and
broadcasts as TensorEngine matmuls with fp32 PSUM accumulation.
"""

import sys

if "/opt/trn_rl_repo" not in sys.path:
    sys.path.insert(0, "/opt/trn_rl_repo")

import numpy as np
import ml_dtypes

FP8 = ml_dtypes.float8_e4m3

from concourse import bacc, mybir
import concourse.bass as bass
import concourse.tile as tile
from concourse.masks import make_identity

BF16 = ml_dtypes.bfloat16

T = 524288
D = 256
N_SEG = 8192
N_CORES = 8
SEGB = 128                      # segments per block
N_BLOCKS = N_SEG // SEGB        # 64 total
BLOCKS_PER_CORE = N_BLOCKS // N_CORES  # 8
DA = D + 2                      # emb + ones column + pad (even stride)
OUT_CHUNK = 32                  # tiles per output staging DMA


def build_nc(NT: int, repeat: int = 1):
    """Build the per-core Bass graph. NT = 128-token tiles per block.
    NT must be even (pairs of tiles share one PSUM bank)."""
    assert NT % 2 == 0
    nc = bacc.Bacc()
    fp32 = mybir.dt.float32
    bf16 = mybir.dt.bfloat16
    B = BLOCKS_PER_CORE
    ACT = mybir.ActivationFunctionType

    emb_ext = nc.declare_dram_parameter("emb", [B * 128, NT * DA], bf16, isOutput=False)
    fp8 = mybir.dt.float8e4
    a_ext = nc.declare_dram_parameter("amat", [B * 128, NT * 128], fp8, isOutput=False)
    at_ext = nc.declare_dram_parameter("atmat", [B * 128, NT * 128], fp8, isOutput=False)
    w_ext = nc.declare_dram_parameter("w", [128, 2 * D], fp32, isOutput=False)
    out_ext = nc.declare_dram_parameter("out", [B * 128, NT * D], bf16, isOutput=True)

    n_chunks = (NT + OUT_CHUNK - 1) // OUT_CHUNK

    with tile.TileContext(nc) as tc:
        with (
            tc.tile_pool(name="const", bufs=1) as const_pool,
            tc.tile_pool(name="emb", bufs=2) as emb_pool,
            tc.tile_pool(name="amat", bufs=2) as a_pool,
            tc.tile_pool(name="atmat", bufs=2) as at_pool,
            tc.tile_pool(name="small", bufs=2) as small_pool,
            tc.tile_pool(name="prodscr", bufs=3) as prod_pool,
            tc.tile_pool(name="redscr", bufs=3) as red_pool,
            tc.tile_pool(name="ascaled", bufs=3) as as_pool,
            tc.tile_pool(name="outstage", bufs=2) as out_pool,
            tc.tile_pool(name="ps_segE", bufs=1, space="PSUM") as ps_segE,
            tc.tile_pool(name="ps_tr", bufs=1, space="PSUM") as ps_tr,
            tc.tile_pool(name="ps_tg", bufs=1, space="PSUM") as ps_tg,
            tc.tile_pool(name="ps_tgtok", bufs=2, space="PSUM") as ps_tgtok,
            tc.tile_pool(name="ps_rep", bufs=1, space="PSUM") as ps_rep,
            tc.tile_pool(name="ps_out", bufs=2, space="PSUM") as ps_out,
        ):
            w_sb = const_pool.tile([128, 2 * D], fp32)
            nc.sync.dma_start(out=w_sb[:], in_=w_ext[:, :])
            ident = const_pool.tile([128, 128], fp32)
            make_identity(nc, ident[:])

            for rep_i in range(repeat):
                for b in range(B):
                    # ---- load block ----
                    emb_sb = emb_pool.tile([128, NT * DA], bf16)
                    nc.sync.dma_start(out=emb_sb[:], in_=emb_ext[b * 128:(b + 1) * 128, :])
                    a_sb = a_pool.tile([128, NT * 128], fp8)
                    nc.sync.dma_start(out=a_sb[:], in_=a_ext[b * 128:(b + 1) * 128, :])
                    at_sb = at_pool.tile([128, NT * 128], fp8)
                    nc.sync.dma_start(out=at_sb[:], in_=at_ext[b * 128:(b + 1) * 128, :])

                    # ---- P1: segE[s, 0:256] = sum emb, segE[s, 256] = count ----
                    segE = ps_segE.tile([128, DA], fp32)
                    for k in range(NT):
                        nc.tensor.matmul(
                            segE[:],
                            lhsT=a_sb[:, k * 128:(k + 1) * 128],
                            rhs=emb_sb[:, k * DA:(k + 1) * DA],
                            start=(k == 0),
                            stop=(k == NT - 1),
                        )

                    # ---- epilogue: tg = tanh((segE/max(cnt,1)) @ W) ----
                    cnt = small_pool.tile([128, 1], fp32, tag="cnt")
                    nc.vector.tensor_scalar_max(cnt[:], segE[:, D:D + 1], 1.0)
                    inv = small_pool.tile([128, 1], fp32, tag="inv")
                    nc.vector.reciprocal(inv[:], cnt[:])
                    segmean = small_pool.tile([128, D], fp32, tag="segmean")
                    nc.vector.tensor_scalar_mul(segmean[:], segE[:, 0:D], inv[:, :])

                    trp = ps_tr.tile([128, D], fp32)
                    nc.tensor.transpose(trp[:, 0:128], segmean[:, 0:128], ident[:])
                    nc.tensor.transpose(trp[:, 128:256], segmean[:, 128:256], ident[:])
                    segmean_t = small_pool.tile([128, D], fp32, tag="segmeant")
                    nc.vector.tensor_copy(segmean_t[:], trp[:])

                    tgp = ps_tg.tile([128, D], fp32)
                    for h in range(2):
                        nc.tensor.matmul(
                            tgp[:],
                            lhsT=segmean_t[:, h * 128:(h + 1) * 128],
                            rhs=w_sb[:, h * D:(h + 1) * D],
                            start=(h == 0),
                            stop=(h == 1),
                        )
                    tg_sb = small_pool.tile([128, D], bf16, tag="tg")
                    nc.scalar.activation(tg_sb[:], tgp[:], ACT.Tanh)

                    # ---- P2a: dots[t] = sum_d emb[t,d] * tg[seg(t),d] ----
                    dots = small_pool.tile([128, NT], fp32, tag="dots")
                    for k in range(0, NT, 2):
                        ttp = ps_tgtok.tile([128, 2 * D], fp32)
                        for t in range(2):
                            nc.tensor.matmul(
                                ttp[:, t * D:(t + 1) * D],
                                lhsT=at_sb[:, (k + t) * 128:(k + t + 1) * 128],
                                rhs=tg_sb[:],
                                start=True, stop=True,
                            )
                        pscr = prod_pool.tile([128, 2 * D], bf16)
                        emb2 = emb_sb[:, k * DA:(k + 2) * DA].rearrange(
                            "p (t c) -> p t c", t=2)[:, :, 0:D]
                        nc.vector.tensor_tensor(
                            out=pscr[:].rearrange("p (t c) -> p t c", t=2),
                            in0=emb2,
                            in1=ttp[:].rearrange("p (t c) -> p t c", t=2),
                            op=mybir.AluOpType.mult,
                        )
                        for t in range(2):
                            rscr = red_pool.tile([128, D], bf16)
                            nc.scalar.activation(
                                rscr[:], pscr[:, t * D:(t + 1) * D], ACT.Copy,
                                accum_out=dots[:, k + t:k + t + 1])
                    sig = small_pool.tile([128, NT], fp32, tag="sig")
                    nc.scalar.activation(sig[:], dots[:], ACT.Sigmoid)

                    # ---- P2b: rep[s] = (A*sig)^T @ emb ----
                    repp = ps_rep.tile([128, D], fp32)
                    for k in range(NT):
                        a_scaled = as_pool.tile([128, 128], bf16)
                        nc.vector.tensor_scalar_mul(
                            a_scaled[:], a_sb[:, k * 128:(k + 1) * 128],
                            sig[:, k:k + 1])
                        nc.tensor.matmul(
                            repp[:],
                            lhsT=a_scaled[:],
                            rhs=emb_sb[:, k * DA:k * DA + D],
                            start=(k == 0),
                            stop=(k == NT - 1),
                        )
                    rep_sb = small_pool.tile([128, D], bf16, tag="rep")
                    nc.vector.tensor_copy(rep_sb[:], repp[:])

                    # ---- P3: out[t] = rep[seg(t)] ----
                    for c in range(n_chunks):
                        k0 = c * OUT_CHUNK
                        k1 = min(k0 + OUT_CHUNK, NT)
                        ostage = out_pool.tile([128, OUT_CHUNK * D], bf16)
                        for k in range(k0, k1, 2):
                            outp = ps_out.tile([128, 2 * D], fp32)
                            for t in range(2):
                                nc.tensor.matmul(
                                    outp[:, t * D:(t + 1) * D],
                                    lhsT=at_sb[:, (k + t) * 128:(k + t + 1) * 128],
                                    rhs=rep_sb[:],
                                    start=True, stop=True,
                                )
                            if (k // 2) % 10 < 3:
                                nc.vector.tensor_copy(
                                    ostage[:, (k - k0) * D:(k - k0 + 2) * D], outp[:])
                            else:
                                nc.scalar.activation(
                                    ostage[:, (k - k0) * D:(k - k0 + 2) * D], outp[:],
                                    ACT.Copy)
                        nc.scalar.dma_start(
                            out=out_ext[b * 128:(b + 1) * 128, k0 * D:k1 * D],
                            in_=ostage[:, 0:(k1 - k0) * D],
                        )
    nc.finalize()
    return nc


def prep_inputs(embedding, W, obj_to_img):
    """Host-side shard + layout. Returns (in_maps, meta)."""
    emb = np.asarray(embedding, dtype=np.float32)
    W = np.asarray(W, dtype=np.float32)
    obj = np.asarray(obj_to_img).astype(np.int64)

    bounds = np.searchsorted(obj, np.arange(0, N_SEG + 1, SEGB))
    cnts = np.diff(bounds)                     # tokens per block [64]
    NT = int(np.ceil(max(int(cnts.max()), 1) / 128.0))
    if NT % 2:
        NT += 1
    L = NT * 128

    emb_bf = emb.astype(BF16)
    w_in = np.ascontiguousarray(W.reshape(2, 128, D).transpose(1, 0, 2).reshape(128, 2 * D))

    idx = np.arange(L).reshape(128, NT)        # p, k -> p*NT + k
    in_maps = []
    meta = {"L": L, "NT": NT, "bounds": bounds, "cnts": cnts}
    for core in range(N_CORES):
        emb_c = np.zeros((BLOCKS_PER_CORE, 128, NT * DA), dtype=BF16)
        a_c = np.zeros((BLOCKS_PER_CORE, 128, NT * 128), dtype=FP8)
        at_c = np.zeros((BLOCKS_PER_CORE, 128, NT * 128), dtype=FP8)
        for bi in range(BLOCKS_PER_CORE):
            blk = core * BLOCKS_PER_CORE + bi
            start, cnt = int(bounds[blk]), int(cnts[blk])
            valid = idx < cnt                   # [128, NT]
            src = start + np.minimum(idx, max(cnt - 1, 0))
            eb = np.zeros((128, NT, DA), dtype=BF16)
            eb[:, :, :D] = np.where(valid[:, :, None], emb_bf[src], BF16(0))
            eb[:, :, D] = valid.astype(BF16)
            emb_c[bi] = eb.reshape(128, NT * DA)
            segloc = np.where(valid, obj[src] - blk * SEGB, 999)  # [128, NT]
            a_blk = (segloc[:, :, None] == np.arange(SEGB)[None, None, :])  # [p,k,s]
            a_c[bi] = a_blk.astype(FP8).reshape(128, NT * 128)
            at_c[bi] = np.ascontiguousarray(
                a_blk.transpose(2, 1, 0)).astype(FP8).reshape(128, NT * 128)
        in_maps.append({
            "emb": emb_c.reshape(BLOCKS_PER_CORE * 128, NT * DA),
            "amat": a_c.reshape(BLOCKS_PER_CORE * 128, NT * 128),
            "atmat": at_c.reshape(BLOCKS_PER_CORE * 128, NT * 128),
            "w": w_in,
        })
    return in_maps, meta


def unshard_output(core_outs, meta):
    """core_outs: list over cores of [B*128, NT*D] (bf16). -> [T, D] f32."""
    L, NT = meta["L"], meta["NT"]
    bounds, cnts = meta["bounds"], meta["cnts"]
    out = np.empty((T, D), dtype=np.float32)
    idx = np.arange(L).reshape(128, NT)
    for core in range(N_CORES):
        o = np.asarray(core_outs[core]).astype(np.float32)
        o = o.reshape(BLOCKS_PER_CORE, 128, NT, D)
        for bi in range(BLOCKS_PER_CORE):
            blk = core * BLOCKS_PER_CORE + bi
            start, cnt = int(bounds[blk]), int(cnts[blk])
            valid = idx < cnt
            p_i, k_i = np.nonzero(valid)
            out[start + idx[valid]] = o[bi, p_i, k_i]
    return out


def kernel(embedding, W, obj_to_img, num_segments):
    assert int(num_segments) == N_SEG
    in_maps, meta = prep_inputs(embedding, W, obj_to_img)
    nc = build_nc(meta["NT"])

    from concourse.bass_utils import run_bass_kernel_spmd
    res = run_bass_kernel_spmd(nc, in_maps, list(range(N_CORES)))
    core_outs = [res.results[i]["out"] for i in range(N_CORES)]
    return unshard_output(core_outs, meta)
